# revision 30
# baseline (speedup 1.0000x reference)
"""Trainium2 Bass kernel for nn_B_NNs_34789235097695.

Problem: per batch element b (B=262144):
    y   = MLP(s_Ddot[b])  (3 -> 128 -> 128 -> 128 -> 3, tanh, fp32)
    K   = diag geometry from (q[b], s[b])
    A   = 3x3 geometry matrix from (q[b], s[b])
    out = Kdiag * solve(A, y + b3)        -> [B, 3, 1]

Strategy (8 cores, pure data parallel, 32768 batch rows per core):
  - MLP on PE in "hidden-on-partitions" layout: psum = W^T @ xT, chunks of
    1536 batch columns (3 matmuls of N=512 per layer per chunk), fp32r
    (full-rate fp32 mode) for layers 0-2, plain fp32 for the tiny layer 3.
  - tanh on ScalarE reading PSUM directly with fused per-partition bias.
    (ACT is the bottleneck engine: 3*128*32768 = 12.6M tanh/core.)
  - Layer 3 uses h3 slices as the *stationary* operand so the MLP output
    lands batch-on-partitions ([128, 3] per 128-batch slice) — the layout
    the elementwise 3x3 Cramer solve on VectorE wants.
  - Geometry (sin/cos polynomials — q in [0,1) — Kdiag, A, cofactors, det,
    reciprocal) entirely on VectorE in batch-on-partition "order B" layout,
    overlapped under the ACT tanh stream.
  - A small PE-transpose pass converts the MLP output from order A
    (b = f*128 + p) to order B (b = p*256 + f) to meet the geometry layout.
  - q/s/out move as 3KB-contiguous runs; s_Ddot is transposed host-side
    during sharding so layer-0 rhs loads are 3 big runs per chunk.

Self-contained: hardcodes all shapes; needs only /opt/trn_rl_repo (the
container's Bass runtime) and the axon-tunneled NeuronCores.
"""

import sys

for _p in ("/opt/trn_rl_repo", "/root/.axon_site/_ro/trn_rl_repo"):
    if _p not in sys.path:
        sys.path.append(_p)

import numpy as np

B_FULL = 262144
N_CORES = 8
BC = B_FULL // N_CORES          # 32768 batch rows per core
F = BC // 128                   # 256 free columns in geometry layout
H = 128

RB = 0.06                       # BASE_RADIUS
RE = 0.045                      # END_EFFECTOR_RADIUS
LA = 0.176                      # LOWER_ARM_LENGTH

MM_DTYPE = "f16"                # "f16" (1 cyc/row) | "f32r" | "f32"

_alpha = np.deg2rad(np.array([-30.0, 90.0, 210.0], np.float32))
CA = [float(v) for v in np.cos(_alpha)]
SA = [float(v) for v in np.sin(_alpha)]

# sin (odd, t=x^2): c1..c9 ; cos (even): d0..d5   -- for q in [0, 1)
_SC = [1.0, -1.0 / 6, 1.0 / 120, -1.0 / 5040, 1.0 / 362880]
_CC = [1.0, -0.5, 1.0 / 24, -1.0 / 720, 1.0 / 40320, -1.0 / 3628800]


CHUNK = 1024                    # 2 PSUM banks per stage tile


def _chunks():
    assert BC % CHUNK == 0
    return [(i * CHUNK, CHUNK) for i in range(BC // CHUNK)]


def _emit(nc, tc, ctx):
    import concourse.bass as bass
    from concourse import mybir

    f32 = mybir.dt.float32
    ALU = mybir.AluOpType
    ACTF = mybir.ActivationFunctionType

    # dtype used along the matmul operand chain
    fmm = {
        "f16": mybir.dt.float16,
        "f32r": mybir.dt.float32r,
        "f32": f32,
    }[MM_DTYPE]

    # ---------------- DRAM tensors (per-core shapes) ----------------
    q_d = nc.dram_tensor("q", [BC, 3], f32, kind="ExternalInput").ap()
    s_d = nc.dram_tensor("s", [BC, 3], f32, kind="ExternalInput").ap()
    sddT_d = nc.dram_tensor("sddT", [3, BC], fmm, kind="ExternalInput").ap()
    W_d = [
        nc.dram_tensor("W0", [3, H], fmm, kind="ExternalInput").ap(),
        nc.dram_tensor("W1", [H, H], fmm, kind="ExternalInput").ap(),
        nc.dram_tensor("W2", [H, H], fmm, kind="ExternalInput").ap(),
        nc.dram_tensor("W3", [H, 3], fmm, kind="ExternalInput").ap(),
    ]
    b_d = [
        nc.dram_tensor("b0", [H], f32, kind="ExternalInput").ap(),
        nc.dram_tensor("b1", [H], f32, kind="ExternalInput").ap(),
        nc.dram_tensor("b2", [H], f32, kind="ExternalInput").ap(),
        nc.dram_tensor("b3", [3], f32, kind="ExternalInput").ap(),
    ]
    out_d = nc.dram_tensor("out", [BC, 3], f32, kind="ExternalOutput").ap()

    # ---------------- pools ----------------
    singles = ctx.enter_context(tc.tile_pool(name="singles", bufs=1))
    geo = ctx.enter_context(tc.tile_pool(name="geo", bufs=1))
    pool_in = ctx.enter_context(tc.tile_pool(name="pool_in", bufs=4))
    pool_h = ctx.enter_context(tc.tile_pool(name="pool_h", bufs=6))
    pool_stg = ctx.enter_context(tc.tile_pool(name="pool_stg", bufs=3))
    # 3 stage tags (mm0/mm1/mm2) x 1 buf x 2 banks + l3 2 x 1 bank = 8 banks
    psum_mm = ctx.enter_context(tc.tile_pool(name="psum_mm", bufs=1, space="PSUM"))
    psum_l3 = ctx.enter_context(tc.tile_pool(name="psum_l3", bufs=2, space="PSUM"))

    # ---------------- PE warm-up burst ----------------
    # HAM un-throttles the PE (1.2 -> 2.4 GHz) only after a ~3.4us window of
    # dense activity; once warm, the steady pipeline keeps it warm. Burn a
    # few us of dummy matmuls during the input-DMA prologue so the real
    # stream starts at full clock.
    warm = singles.tile([128, 128], fmm, name="warm", tag="warm")
    nc.vector.memset(warm, 0.0)
    wpsum = psum_l3.tile([3, 512], f32, name="wpsum", tag="l3")
    for _ in range(50):
        nc.tensor.matmul(wpsum[:, 0:128], warm[:, 0:3], warm,
                         start=True, stop=True)
    # dummy activation pulls the ~1.3us tanh ACT_TABLE_LOAD into the prologue
    wact = singles.tile([128, 1], f32, name="wact", tag="wact")
    nc.scalar.activation(wact, warm[:, 0:1], ACTF.Tanh)

    # ---------------- constants / weights in SBUF ----------------
    # Critical-path first: w0 + b0 + the first input chunk feed the first
    # tanh; everything else hides under the pipeline.
    w_sb = [None] * 4
    b_sb = [None] * 3

    def load_w(i):
        w = singles.tile(list(W_d[i].shape), W_d[i].dtype, name=f"w{i}sb",
                         tag=f"w{i}sb")
        nc.sync.dma_start(out=w, in_=W_d[i])
        w_sb[i] = w

    def load_b(i):
        b = singles.tile([H, 1], f32, name=f"b{i}sb", tag=f"b{i}sb")
        nc.sync.dma_start(out=b, in_=b_d[i].rearrange("(p one) -> p one", one=1))
        b_sb[i] = b

    load_w(0)
    load_b(0)

    # b3 broadcast to all partitions: [128, 3]
    b3bc = singles.tile([128, 3], f32, name="b3bc", tag="b3bc")

    def load_rest():
        for i in (1, 2, 3):
            load_w(i)
            if i < 3:
                load_b(i)
        nc.gpsimd.dma_start(
            out=b3bc,
            in_=bass.AP(tensor=b_d[3].tensor, offset=0, ap=[[0, 128], [1, 3]]),
        )

    # interleaved q/s in order B: partition p holds rows [p*F, (p+1)*F).
    # Loads are deferred into the geometry stream so they don't delay the
    # first MLP chunk's input DMA (ACT prologue latency).
    iq = singles.tile([128, F, 3], f32, name="iq", tag="iq")
    is_ = singles.tile([128, F, 3], f32, name="is_", tag="is_")

    # MLP output in order B, comp-major: yB[p, 256*c + f] = y[p*256 + f, c]
    yB = singles.tile([128, 3 * F], f32, name="yB", tag="yB")

    # ---------------- geometry op list (drained between chunks) ----------
    G = {}  # name -> AP

    def gt(name):
        t = geo.tile([128, F], f32, name=name, tag=name)
        G[name] = t
        return t

    geo_ops = []

    def deferred(fn):
        geo_ops.append(fn)

    vec = nc.vector

    def op_load_iq():
        # GpSimd SWDGE queue — keeps the Sync HWDGE queue free for the
        # latency-critical per-chunk input loads.
        nc.gpsimd.dma_start(out=iq, in_=q_d.rearrange("(p f) c -> p f c", p=128))

    def op_load_is():
        nc.gpsimd.dma_start(out=is_, in_=s_d.rearrange("(p f) c -> p f c", p=128))

    deferred(op_load_iq)
    deferred(op_load_is)

    def emit_trig(c):
        x = iq[:, :, c]

        def op_t():
            t = gt(f"t{c}")
            vec.tensor_mul(t, x, x)
        deferred(op_t)

        def op_sin():
            t = G[f"t{c}"]
            c1, c3, c5, c7, c9 = _SC
            w = gt(f"sw{c}")
            vec.scalar_tensor_tensor(w, t, c7 / c9, t, op0=ALU.add, op1=ALU.mult)
            vec.scalar_tensor_tensor(w, w, c5 / c9, t, op0=ALU.add, op1=ALU.mult)
            vec.scalar_tensor_tensor(w, w, c3 / c9, t, op0=ALU.add, op1=ALU.mult)
            vec.tensor_scalar(w, w, c9, 1.0, op0=ALU.mult, op1=ALU.add)
            sq = gt(f"sq{c}")
            vec.tensor_mul(sq, w, x)
        deferred(op_sin)

        def op_cos():
            t = G[f"t{c}"]
            d0, d1, d2, d3, d4, d5 = _CC
            w = gt(f"cw{c}")
            vec.scalar_tensor_tensor(w, t, d4 / d5, t, op0=ALU.add, op1=ALU.mult)
            vec.scalar_tensor_tensor(w, w, d3 / d5, t, op0=ALU.add, op1=ALU.mult)
            vec.scalar_tensor_tensor(w, w, d2 / d5, t, op0=ALU.add, op1=ALU.mult)
            vec.scalar_tensor_tensor(w, w, d1 / d5, t, op0=ALU.add, op1=ALU.mult)
            cq = gt(f"cq{c}")
            vec.tensor_scalar(cq, w, d5, 1.0, op0=ALU.mult, op1=ALU.add)
        deferred(op_cos)

    def emit_kdiag_a(c):
        s0, s1, s2 = is_[:, :, 0], is_[:, :, 1], is_[:, :, 2]

        def op_k():
            sq, cq = G[f"sq{c}"], G[f"cq{c}"]
            u = gt(f"ku{c}")
            vec.tensor_scalar(u, s0, CA[c], RB - RE, op0=ALU.mult, op1=ALU.add)
            vec.scalar_tensor_tensor(u, s1, SA[c], u, op0=ALU.mult, op1=ALU.add)
            vec.tensor_mul(u, u, sq)
            w = gt(f"kw{c}")
            vec.tensor_mul(w, s2, cq)
            k = gt(f"K{c}")
            vec.tensor_sub(k, u, w)
        deferred(op_k)

        def op_a():
            cq = G[f"cq{c}"]
            dR = RE - RB
            a0 = gt(f"a0{c}")
            vec.tensor_scalar(a0, cq, -LA * CA[c], dR * CA[c],
                              op0=ALU.mult, op1=ALU.add)
            vec.tensor_add(a0, a0, s0)
            a1 = gt(f"a1{c}")
            vec.tensor_scalar(a1, cq, -LA * SA[c], dR * SA[c],
                              op0=ALU.mult, op1=ALU.add)
            vec.tensor_add(a1, a1, s1)
            a2 = gt(f"a2{c}")
            vec.scalar_tensor_tensor(a2, cq, -LA, s2, op0=ALU.mult, op1=ALU.add)
        deferred(op_a)

    for c in range(3):
        emit_trig(c)
    for c in range(3):
        emit_kdiag_a(c)

    # cofactors C[i][j] of entry (i,j); adj = C^T ; x_i = sum_j C[j][i]*r_j
    COF = [
        ((0, 0), (1, 1), (2, 2), (1, 2), (2, 1)),
        ((0, 1), (1, 2), (2, 0), (1, 0), (2, 2)),
        ((0, 2), (1, 0), (2, 1), (1, 1), (2, 0)),
        ((1, 0), (0, 2), (2, 1), (0, 1), (2, 2)),
        ((1, 1), (0, 0), (2, 2), (0, 2), (2, 0)),
        ((1, 2), (0, 1), (2, 0), (0, 0), (2, 1)),
        ((2, 0), (0, 1), (1, 2), (0, 2), (1, 1)),
        ((2, 1), (0, 2), (1, 0), (0, 0), (1, 2)),
        ((2, 2), (0, 0), (1, 1), (0, 1), (1, 0)),
    ]

    # cofactors on the otherwise-idle GpSimd engine (SBUF-only elementwise)
    def emit_cof(spec):
        (ci, cj), (pi, pj), (pk, pl), (ni, nj), (nk, nl) = spec

        def op():
            gp = nc.gpsimd
            m1 = gt(f"cm1_{ci}{cj}")
            gp.tensor_mul(m1, G[f"a{pi}{pj}"], G[f"a{pk}{pl}"])
            m2 = gt(f"cm2_{ci}{cj}")
            gp.tensor_mul(m2, G[f"a{ni}{nj}"], G[f"a{nk}{nl}"])
            cc = gt(f"C{ci}{cj}")
            gp.tensor_sub(cc, m1, m2)
        deferred(op)

    for spec in COF:
        emit_cof(spec)

    def op_det():
        # GpSimd, like the cofactors it depends on — keeps the strict-FIFO
        # DVE queue free of cross-engine waits that would block the L3
        # staging copies behind them.
        gp = nc.gpsimd
        m1 = gt("dm1")
        gp.tensor_mul(m1, G["a00"], G["C00"])
        m2 = gt("dm2")
        gp.tensor_mul(m2, G["a01"], G["C01"])
        gp.tensor_add(m1, m1, m2)
        gp.tensor_mul(m2, G["a02"], G["C02"])
        det = gt("det")
        gp.tensor_add(det, m1, m2)
    deferred(op_det)

    def op_rdet():
        # emitted after the chunk loop: keeps the reciprocal + Krd muls out
        # of the DVE queue's steady-state stream (det is long done by then).
        rdet = gt("rdet")
        vec.reciprocal(rdet, G["det"])
        for c in range(3):
            krd = gt(f"Krd{c}")
            vec.tensor_mul(krd, G[f"K{c}"], rdet)

    # ---------------- MLP chunks: 3-stage skewed software pipeline -------
    # ACT is the bottleneck engine and its queue is strict-FIFO, so tanh
    # instructions are emitted in the order T0(i), T1(i-1), T2(i-2): the
    # PE work between dependent tanhs of one chunk is hidden under the
    # other chunks' tanhs, keeping ACT (and PE, for HAM warmth) dense.
    chunks = _chunks()
    n_chunks = len(chunks)
    n_iters = n_chunks + 2
    per_gap = (len(geo_ops) + n_iters - 1) // n_iters

    PS = {}   # (stage, chunk) -> psum tile
    HT = {}   # (stage, chunk) -> h tile

    def st_dma(ci):
        off, S = chunks[ci]
        sddc = pool_in.tile([3, S], fmm, name=f"sdd_{ci}", tag="sdd")
        nc.sync.dma_start(out=sddc, in_=sddT_d[:, off:off + S])
        HT[("x", ci)] = sddc

    def st_mm(layer, ci):
        _, S = chunks[ci]
        nS = S // 512
        src = HT[("x", ci)] if layer == 0 else HT[(layer - 1, ci)]
        ps = psum_mm.tile([128, S], f32, name=f"ps{layer}_{ci}",
                          tag=f"mm{layer}")
        for k in range(nS):
            nc.tensor.matmul(ps[:, 512 * k:512 * (k + 1)], w_sb[layer],
                             src[:, 512 * k:512 * (k + 1)],
                             start=True, stop=True)
        PS[(layer, ci)] = ps

    def st_tanh(layer, ci):
        _, S = chunks[ci]
        h = pool_h.tile([128, S], fmm, name=f"h{layer}_{ci}", tag="h")
        nc.scalar.activation(h, PS[(layer, ci)], ACTF.Tanh, bias=b_sb[layer])
        HT[(layer, ci)] = h
        del PS[(layer, ci)]

    GRP = 4                       # chunks per staging group / reshape DMA
    STG = {}

    def st_l3(ci):
        off, S = chunks[ci]
        nS = S // 512
        g = ci // GRP
        if g not in STG:
            STG[g] = pool_stg.tile([3, GRP * CHUNK], f32, name=f"stg_{g}",
                                   tag="stg")
        stg = STG[g]
        goff = (ci % GRP) * CHUNK
        h3 = HT[(2, ci)]
        for k in range(nS):
            psl3 = psum_l3.tile([3, 512], f32, name=f"l3_{ci}_{k}", tag="l3")
            nc.tensor.matmul(psl3, w_sb[3], h3[:, 512 * k:512 * (k + 1)],
                             start=True, stop=True)
            vec.tensor_copy(stg[:, goff + 512 * k:goff + 512 * (k + 1)], psl3)
        if ci % GRP == GRP - 1:
            nP = GRP * CHUNK // F
            p0 = (off + S - GRP * CHUNK) // F
            for c in range(3):
                nc.sync.dma_start(
                    out=yB[p0:p0 + nP, F * c:F * (c + 1)],
                    in_=stg[c:c + 1, :].rearrange("one (p f) -> one p f", f=F),
                )
            del STG[g]

    st_dma(0)
    load_rest()
    st_mm(0, 0)
    st_dma(1)
    for i in range(n_iters):
        if i + 2 < n_chunks:
            st_dma(i + 2)
        if i + 1 < n_chunks:
            st_mm(0, i + 1)
        if i < n_chunks:
            st_tanh(0, i)
            st_mm(1, i)
        if 0 <= i - 1 < n_chunks:
            st_tanh(1, i - 1)
            st_mm(2, i - 1)
        if 0 <= i - 2 < n_chunks:
            st_tanh(2, i - 2)
            st_l3(i - 2)
        for _ in range(per_gap):
            if geo_ops:
                geo_ops.pop(0)()

    while geo_ops:
        geo_ops.pop(0)()
    op_rdet()

    # ---------------- r_c = yB_c + b3[c] --------------------------------
    # DVE handles components 0-1, GpSimd component 2 — the tail combine is
    # the last serial stretch after the final tanh, so split it across the
    # two elementwise engines.
    for c in range(3):
        eng = nc.gpsimd if c == 2 else vec
        rb = gt(f"r{c}")
        eng.tensor_scalar(rb, yB[:, F * c:F * (c + 1)], b3bc[:, c:c + 1],
                          None, op0=ALU.add)

    # ---------------- final combine: out = Krd * (C^T r) ----------------
    out_int = singles.tile([128, F, 3], f32, name="out_int", tag="out_int")
    for i in range(3):
        eng = nc.gpsimd if i == 2 else vec
        m1 = gt(f"fm1_{i}")
        eng.tensor_mul(m1, G[f"C0{i}"], G["r0"])
        m2 = gt(f"fm2_{i}")
        eng.tensor_mul(m2, G[f"C1{i}"], G["r1"])
        eng.tensor_add(m1, m1, m2)
        eng.tensor_mul(m2, G[f"C2{i}"], G["r2"])
        eng.tensor_add(m1, m1, m2)
        eng.tensor_mul(out_int[:, :, i], m1, G[f"Krd{i}"])

    nc.sync.dma_start(out=out_d.rearrange("(p f) c -> p f c", p=128), in_=out_int)


def build():
    """Build the per-core Bass program (same program for all 8 cores)."""
    from contextlib import ExitStack

    import concourse.bacc as bacc
    import concourse.tile as tile

    nc = bacc.Bacc(trn_type="TRN2", target_bir_lowering=False, debug=False)
    with tile.TileContext(nc) as tc:
        with ExitStack() as ctx:
            _emit(nc, tc, ctx)
    nc.compile()
    return nc


_NC_CACHE = []


def _shard_inputs(inputs):
    f32 = np.float32
    fmm = {"f16": np.float16, "f32r": f32, "f32": f32}[MM_DTYPE]
    q = np.ascontiguousarray(np.asarray(inputs["q"], dtype=f32))
    s = np.ascontiguousarray(np.asarray(inputs["s"], dtype=f32))
    sdd = np.asarray(inputs["s_Ddot"], dtype=f32)
    weights = {}
    for k in ("W0", "W1", "W2", "W3"):
        weights[k] = np.ascontiguousarray(np.asarray(inputs[k], dtype=f32).astype(fmm))
    for k in ("b0", "b1", "b2", "b3"):
        weights[k] = np.ascontiguousarray(np.asarray(inputs[k], dtype=f32))
    in_maps = []
    for c in range(N_CORES):
        sl = slice(c * BC, (c + 1) * BC)
        m = {
            "q": q[sl],
            "s": s[sl],
            "sddT": np.ascontiguousarray(sdd[sl].T).astype(fmm),
        }
        m.update(weights)
        in_maps.append(m)
    return in_maps


def kernel(**inputs) -> np.ndarray:
    from concourse import bass_utils

    if not _NC_CACHE:
        _NC_CACHE.append(build())
    nc = _NC_CACHE[0]

    in_maps = _shard_inputs(inputs)
    res = bass_utils.run_bass_kernel_spmd(nc, in_maps, core_ids=list(range(N_CORES)))
    out = np.concatenate([res.results[c]["out"] for c in range(N_CORES)], axis=0)
    return out.reshape(B_FULL, 3, 1).astype(np.float32)


if __name__ == "__main__":
    nc = build()
    print("built OK")


# revision 35
# speedup vs baseline: 1.1097x; 1.1097x over previous
"""Trainium2 Bass kernel for nn_B_NNs_34789235097695.

Problem: per batch element b (B=262144):
    y   = MLP(s_Ddot[b])  (3 -> 128 -> 128 -> 128 -> 3, tanh, fp32)
    K   = diag geometry from (q[b], s[b])
    A   = 3x3 geometry matrix from (q[b], s[b])
    out = Kdiag * solve(A, y + b3)        -> [B, 3, 1]

Strategy (8 cores, pure data parallel, 32768 batch rows per core):
  - MLP on PE in "hidden-on-partitions" layout: psum = W^T @ xT, chunks of
    1536 batch columns (3 matmuls of N=512 per layer per chunk), fp32r
    (full-rate fp32 mode) for layers 0-2, plain fp32 for the tiny layer 3.
  - tanh on ScalarE reading PSUM directly with fused per-partition bias.
    (ACT is the bottleneck engine: 3*128*32768 = 12.6M tanh/core.)
  - Layer 3 uses h3 slices as the *stationary* operand so the MLP output
    lands batch-on-partitions ([128, 3] per 128-batch slice) — the layout
    the elementwise 3x3 Cramer solve on VectorE wants.
  - Geometry (sin/cos polynomials — q in [0,1) — Kdiag, A, cofactors, det,
    reciprocal) entirely on VectorE in batch-on-partition "order B" layout,
    overlapped under the ACT tanh stream.
  - A small PE-transpose pass converts the MLP output from order A
    (b = f*128 + p) to order B (b = p*256 + f) to meet the geometry layout.
  - q/s/out move as 3KB-contiguous runs; s_Ddot is transposed host-side
    during sharding so layer-0 rhs loads are 3 big runs per chunk.

Self-contained: hardcodes all shapes; needs only /opt/trn_rl_repo (the
container's Bass runtime) and the axon-tunneled NeuronCores.
"""

import sys

for _p in ("/opt/trn_rl_repo", "/root/.axon_site/_ro/trn_rl_repo"):
    if _p not in sys.path:
        sys.path.append(_p)

import numpy as np

B_FULL = 262144
N_CORES = 8
BC = B_FULL // N_CORES          # 32768 batch rows per core
F = BC // 128                   # 256 free columns in geometry layout
H = 128

RB = 0.06                       # BASE_RADIUS
RE = 0.045                      # END_EFFECTOR_RADIUS
LA = 0.176                      # LOWER_ARM_LENGTH

MM_DTYPE = "f16"                # "f16" (1 cyc/row) | "f32r" | "f32"

_alpha = np.deg2rad(np.array([-30.0, 90.0, 210.0], np.float32))
CA = [float(v) for v in np.cos(_alpha)]
SA = [float(v) for v in np.sin(_alpha)]

# sin (odd, t=x^2): c1..c9 ; cos (even): d0..d5   -- for q in [0, 1)
_SC = [1.0, -1.0 / 6, 1.0 / 120, -1.0 / 5040, 1.0 / 362880]
_CC = [1.0, -0.5, 1.0 / 24, -1.0 / 720, 1.0 / 40320, -1.0 / 3628800]


CHUNK = 1024                    # 2 PSUM banks per stage tile


def _chunks():
    assert BC % CHUNK == 0
    return [(i * CHUNK, CHUNK) for i in range(BC // CHUNK)]


def _emit(nc, tc, ctx):
    import concourse.bass as bass
    from concourse import mybir

    f32 = mybir.dt.float32
    ALU = mybir.AluOpType
    ACTF = mybir.ActivationFunctionType

    # dtype used along the matmul operand chain
    fmm = {
        "f16": mybir.dt.float16,
        "f32r": mybir.dt.float32r,
        "f32": f32,
    }[MM_DTYPE]

    # ---------------- DRAM tensors (per-core shapes) ----------------
    q_d = nc.dram_tensor("q", [BC, 3], f32, kind="ExternalInput").ap()
    s_d = nc.dram_tensor("s", [BC, 3], f32, kind="ExternalInput").ap()
    sddT_d = nc.dram_tensor("sddT", [3, BC], fmm, kind="ExternalInput").ap()
    W_d = [
        nc.dram_tensor("W0", [3, H], fmm, kind="ExternalInput").ap(),
        nc.dram_tensor("W1", [H, H], fmm, kind="ExternalInput").ap(),
        nc.dram_tensor("W2", [H, H], fmm, kind="ExternalInput").ap(),
        nc.dram_tensor("W3", [H, 3], fmm, kind="ExternalInput").ap(),
    ]
    b_d = [
        nc.dram_tensor("b0", [H], f32, kind="ExternalInput").ap(),
        nc.dram_tensor("b1", [H], f32, kind="ExternalInput").ap(),
        nc.dram_tensor("b2", [H], f32, kind="ExternalInput").ap(),
        nc.dram_tensor("b3", [3], f32, kind="ExternalInput").ap(),
    ]
    out_d = nc.dram_tensor("out", [BC, 3], f32, kind="ExternalOutput").ap()

    # ---------------- pools ----------------
    singles = ctx.enter_context(tc.tile_pool(name="singles", bufs=1))
    geo = ctx.enter_context(tc.tile_pool(name="geo", bufs=1))
    pool_in = ctx.enter_context(tc.tile_pool(name="pool_in", bufs=4))
    pool_h = ctx.enter_context(tc.tile_pool(name="pool_h", bufs=6))
    pool_stg = ctx.enter_context(tc.tile_pool(name="pool_stg", bufs=3))
    # 3 stage tags (mm0/mm1/mm2) x 1 buf x 2 banks + l3 2 x 1 bank = 8 banks
    psum_mm = ctx.enter_context(tc.tile_pool(name="psum_mm", bufs=1, space="PSUM"))
    psum_l3 = ctx.enter_context(tc.tile_pool(name="psum_l3", bufs=2, space="PSUM"))

    # ---------------- PE warm-up burst ----------------
    # HAM un-throttles the PE (1.2 -> 2.4 GHz) only after a ~3.4us window of
    # dense activity; once warm, the steady pipeline keeps it warm. Burn a
    # few us of dummy matmuls during the input-DMA prologue so the real
    # stream starts at full clock.
    warm = singles.tile([128, 128], fmm, name="warm", tag="warm")
    nc.vector.memset(warm, 0.0)
    wpsum = psum_l3.tile([3, 512], f32, name="wpsum", tag="l3")
    for _ in range(55):
        nc.tensor.matmul(wpsum[:, 0:128], warm[:, 0:3], warm,
                         start=True, stop=True)
    # dummy activation pulls the ~1.3us tanh ACT_TABLE_LOAD into the prologue
    wact = singles.tile([128, 1], f32, name="wact", tag="wact")
    nc.scalar.activation(wact, warm[:, 0:1], ACTF.Tanh)

    # ---------------- constants / weights in SBUF ----------------
    # Critical-path first: w0 + b0 + the first input chunk feed the first
    # tanh; everything else hides under the pipeline.
    w_sb = [None] * 4
    b_sb = [None] * 3

    def load_w(i):
        w = singles.tile(list(W_d[i].shape), W_d[i].dtype, name=f"w{i}sb",
                         tag=f"w{i}sb")
        nc.sync.dma_start(out=w, in_=W_d[i])
        w_sb[i] = w

    def load_b(i):
        b = singles.tile([H, 1], f32, name=f"b{i}sb", tag=f"b{i}sb")
        nc.sync.dma_start(out=b, in_=b_d[i].rearrange("(p one) -> p one", one=1))
        b_sb[i] = b

    load_w(0)
    load_b(0)

    # b3 broadcast to all partitions: [128, 3]
    b3bc = singles.tile([128, 3], f32, name="b3bc", tag="b3bc")

    def load_rest():
        for i in (1, 2, 3):
            load_w(i)
            if i < 3:
                load_b(i)
        nc.gpsimd.dma_start(
            out=b3bc,
            in_=bass.AP(tensor=b_d[3].tensor, offset=0, ap=[[0, 128], [1, 3]]),
        )

    # interleaved q/s in order B: partition p holds rows [p*F, (p+1)*F).
    # Loads are deferred into the geometry stream so they don't delay the
    # first MLP chunk's input DMA (ACT prologue latency).
    iq = singles.tile([128, F, 3], f32, name="iq", tag="iq")
    is_ = singles.tile([128, F, 3], f32, name="is_", tag="is_")

    # MLP output in order B, comp-major: yB[p, 256*c + f] = y[p*256 + f, c]
    yB = singles.tile([128, 3 * F], f32, name="yB", tag="yB")

    # ---------------- geometry op list (drained between chunks) ----------
    G = {}  # name -> AP

    def gt(name):
        t = geo.tile([128, F], f32, name=name, tag=name)
        G[name] = t
        return t

    geo_ops = []

    def deferred(fn):
        geo_ops.append(fn)

    vec = nc.vector

    def op_load_iq():
        # GpSimd SWDGE queue — keeps the Sync HWDGE queue free for the
        # latency-critical per-chunk input loads.
        nc.gpsimd.dma_start(out=iq, in_=q_d.rearrange("(p f) c -> p f c", p=128))

    def op_load_is():
        nc.gpsimd.dma_start(out=is_, in_=s_d.rearrange("(p f) c -> p f c", p=128))

    deferred(op_load_iq)
    deferred(op_load_is)

    def emit_trig(c):
        x = iq[:, :, c]
        gp = vec

        def op_t():
            t = gt(f"t{c}")
            gp.tensor_mul(t, x, x)
        deferred(op_t)

        def op_sin():
            t = G[f"t{c}"]
            c1, c3, c5, c7, c9 = _SC
            w = gt(f"sw{c}")
            gp.scalar_tensor_tensor(w, t, c7 / c9, t, op0=ALU.add, op1=ALU.mult)
            gp.scalar_tensor_tensor(w, w, c5 / c9, t, op0=ALU.add, op1=ALU.mult)
            gp.scalar_tensor_tensor(w, w, c3 / c9, t, op0=ALU.add, op1=ALU.mult)
            gp.tensor_scalar(w, w, c9, 1.0, op0=ALU.mult, op1=ALU.add)
            sq = gt(f"sq{c}")
            gp.tensor_mul(sq, w, x)
        deferred(op_sin)

        def op_cos():
            t = G[f"t{c}"]
            d0, d1, d2, d3, d4, d5 = _CC
            w = gt(f"cw{c}")
            gp.scalar_tensor_tensor(w, t, d4 / d5, t, op0=ALU.add, op1=ALU.mult)
            gp.scalar_tensor_tensor(w, w, d3 / d5, t, op0=ALU.add, op1=ALU.mult)
            gp.scalar_tensor_tensor(w, w, d2 / d5, t, op0=ALU.add, op1=ALU.mult)
            gp.scalar_tensor_tensor(w, w, d1 / d5, t, op0=ALU.add, op1=ALU.mult)
            cq = gt(f"cq{c}")
            gp.tensor_scalar(cq, w, d5, 1.0, op0=ALU.mult, op1=ALU.add)
        deferred(op_cos)

    def emit_kdiag_a(c):
        s0, s1, s2 = is_[:, :, 0], is_[:, :, 1], is_[:, :, 2]
        gp = vec

        def op_k():
            sq, cq = G[f"sq{c}"], G[f"cq{c}"]
            u = gt(f"ku{c}")
            gp.tensor_scalar(u, s0, CA[c], RB - RE, op0=ALU.mult, op1=ALU.add)
            gp.scalar_tensor_tensor(u, s1, SA[c], u, op0=ALU.mult, op1=ALU.add)
            gp.tensor_mul(u, u, sq)
            w = gt(f"kw{c}")
            gp.tensor_mul(w, s2, cq)
            k = gt(f"K{c}")
            gp.tensor_sub(k, u, w)
        deferred(op_k)

        def op_a():
            cq = G[f"cq{c}"]
            dR = RE - RB
            a0 = gt(f"a0{c}")
            gp.tensor_scalar(a0, cq, -LA * CA[c], dR * CA[c],
                             op0=ALU.mult, op1=ALU.add)
            gp.tensor_add(a0, a0, s0)
            a1 = gt(f"a1{c}")
            gp.tensor_scalar(a1, cq, -LA * SA[c], dR * SA[c],
                             op0=ALU.mult, op1=ALU.add)
            gp.tensor_add(a1, a1, s1)
            a2 = gt(f"a2{c}")
            gp.scalar_tensor_tensor(a2, cq, -LA, s2, op0=ALU.mult, op1=ALU.add)
        deferred(op_a)

    for c in range(3):
        emit_trig(c)
    for c in range(3):
        emit_kdiag_a(c)

    # cofactors C[i][j] of entry (i,j); adj = C^T ; x_i = sum_j C[j][i]*r_j
    COF = [
        ((0, 0), (1, 1), (2, 2), (1, 2), (2, 1)),
        ((0, 1), (1, 2), (2, 0), (1, 0), (2, 2)),
        ((0, 2), (1, 0), (2, 1), (1, 1), (2, 0)),
        ((1, 0), (0, 2), (2, 1), (0, 1), (2, 2)),
        ((1, 1), (0, 0), (2, 2), (0, 2), (2, 0)),
        ((1, 2), (0, 1), (2, 0), (0, 0), (2, 1)),
        ((2, 0), (0, 1), (1, 2), (0, 2), (1, 1)),
        ((2, 1), (0, 2), (1, 0), (0, 0), (1, 2)),
        ((2, 2), (0, 0), (1, 1), (0, 1), (1, 0)),
    ]

    # cofactors on the otherwise-idle GpSimd engine (SBUF-only elementwise)
    def emit_cof(spec):
        (ci, cj), (pi, pj), (pk, pl), (ni, nj), (nk, nl) = spec

        def op():
            gp = nc.gpsimd
            m1 = gt(f"cm1_{ci}{cj}")
            gp.tensor_mul(m1, G[f"a{pi}{pj}"], G[f"a{pk}{pl}"])
            m2 = gt(f"cm2_{ci}{cj}")
            gp.tensor_mul(m2, G[f"a{ni}{nj}"], G[f"a{nk}{nl}"])
            cc = gt(f"C{ci}{cj}")
            gp.tensor_sub(cc, m1, m2)
        deferred(op)

    for spec in COF:
        emit_cof(spec)

    def op_det():
        # GpSimd, like the cofactors it depends on — keeps the strict-FIFO
        # DVE queue free of cross-engine waits that would block the L3
        # staging copies behind them.
        gp = nc.gpsimd
        m1 = gt("dm1")
        gp.tensor_mul(m1, G["a00"], G["C00"])
        m2 = gt("dm2")
        gp.tensor_mul(m2, G["a01"], G["C01"])
        gp.tensor_add(m1, m1, m2)
        gp.tensor_mul(m2, G["a02"], G["C02"])
        det = gt("det")
        gp.tensor_add(det, m1, m2)
    deferred(op_det)

    LATE_COPY = []

    def op_rdet():
        # emitted after the chunk loop; the explicit (non-sync) dep edge on a
        # late L3 staging copy stops the scheduler from hoisting the
        # reciprocal into the steady-state DVE stream, where a wait on the
        # GpSimd det chain would head-of-line-block the PSUM copies.
        from concourse.tile import add_dep_helper

        rdet = gt("rdet")
        ri = vec.reciprocal(rdet, G["det"])
        if LATE_COPY:
            add_dep_helper(ri.ins, LATE_COPY[0].ins, sync=False,
                           reason="rdet after steady-state copies")
        for c in range(3):
            krd = gt(f"Krd{c}")
            vec.tensor_mul(krd, G[f"K{c}"], rdet)

    # ---------------- MLP chunks: 3-stage skewed software pipeline -------
    # ACT is the bottleneck engine and its queue is strict-FIFO, so tanh
    # instructions are emitted in the order T0(i), T1(i-1), T2(i-2): the
    # PE work between dependent tanhs of one chunk is hidden under the
    # other chunks' tanhs, keeping ACT (and PE, for HAM warmth) dense.
    chunks = _chunks()
    n_chunks = len(chunks)
    n_iters = n_chunks + 2
    pace = 12
    per_gap = (len(geo_ops) + pace - 1) // pace

    PS = {}   # (stage, chunk) -> psum tile
    HT = {}   # (stage, chunk) -> h tile

    def st_dma(ci):
        off, S = chunks[ci]
        sddc = pool_in.tile([3, S], fmm, name=f"sdd_{ci}", tag="sdd")
        nc.sync.dma_start(out=sddc, in_=sddT_d[:, off:off + S])
        HT[("x", ci)] = sddc

    def st_mm(layer, ci):
        _, S = chunks[ci]
        nS = S // 512
        src = HT[("x", ci)] if layer == 0 else HT[(layer - 1, ci)]
        ps = psum_mm.tile([128, S], f32, name=f"ps{layer}_{ci}",
                          tag=f"mm{layer}")
        for k in range(nS):
            nc.tensor.matmul(ps[:, 512 * k:512 * (k + 1)], w_sb[layer],
                             src[:, 512 * k:512 * (k + 1)],
                             start=True, stop=True)
        PS[(layer, ci)] = ps

    def st_tanh(layer, ci):
        _, S = chunks[ci]
        h = pool_h.tile([128, S], fmm, name=f"h{layer}_{ci}", tag="h")
        nc.scalar.activation(h, PS[(layer, ci)], ACTF.Tanh, bias=b_sb[layer])
        HT[(layer, ci)] = h
        del PS[(layer, ci)]

    GRP = 4                       # chunks per staging group / reshape DMA
    STG = {}

    def st_l3(ci):
        off, S = chunks[ci]
        nS = S // 512
        g = ci // GRP
        if g not in STG:
            STG[g] = pool_stg.tile([3, GRP * CHUNK], f32, name=f"stg_{g}",
                                   tag="stg")
        stg = STG[g]
        goff = (ci % GRP) * CHUNK
        h3 = HT[(2, ci)]
        for k in range(nS):
            psl3 = psum_l3.tile([3, 512], f32, name=f"l3_{ci}_{k}", tag="l3")
            nc.tensor.matmul(psl3, w_sb[3], h3[:, 512 * k:512 * (k + 1)],
                             start=True, stop=True)
            cp = vec.tensor_copy(stg[:, goff + 512 * k:goff + 512 * (k + 1)],
                                 psl3)
            if ci == 26 and k == 0:
                LATE_COPY.append(cp)
        if ci % GRP == GRP - 1:
            nP = GRP * CHUNK // F
            p0 = (off + S - GRP * CHUNK) // F
            for c in range(3):
                nc.sync.dma_start(
                    out=yB[p0:p0 + nP, F * c:F * (c + 1)],
                    in_=stg[c:c + 1, :].rearrange("one (p f) -> one p f", f=F),
                )
            del STG[g]

    st_dma(0)
    load_rest()
    st_mm(0, 0)
    st_dma(1)
    for i in range(n_iters):
        if i + 2 < n_chunks:
            st_dma(i + 2)
        if i + 1 < n_chunks:
            st_mm(0, i + 1)
        if i < n_chunks:
            st_tanh(0, i)
            st_mm(1, i)
        if 0 <= i - 1 < n_chunks:
            st_tanh(1, i - 1)
            st_mm(2, i - 1)
        if 0 <= i - 2 < n_chunks:
            st_tanh(2, i - 2)
            st_l3(i - 2)
        for _ in range(per_gap):
            if geo_ops:
                geo_ops.pop(0)()

    while geo_ops:
        geo_ops.pop(0)()
    op_rdet()

    # ---------------- r_c = yB_c + b3[c] --------------------------------
    # DVE handles components 0-1, GpSimd component 2 — the tail combine is
    # the last serial stretch after the final tanh, so split it across the
    # two elementwise engines.
    for c in range(3):
        eng = nc.gpsimd if c == 2 else vec
        rb = gt(f"r{c}")
        eng.tensor_scalar(rb, yB[:, F * c:F * (c + 1)], b3bc[:, c:c + 1],
                          None, op0=ALU.add)

    # ---------------- final combine: out = Krd * (C^T r) ----------------
    out_int = singles.tile([128, F, 3], f32, name="out_int", tag="out_int")
    for i in range(3):
        eng = nc.gpsimd if i == 2 else vec
        m1 = gt(f"fm1_{i}")
        eng.tensor_mul(m1, G[f"C0{i}"], G["r0"])
        m2 = gt(f"fm2_{i}")
        eng.tensor_mul(m2, G[f"C1{i}"], G["r1"])
        eng.tensor_add(m1, m1, m2)
        eng.tensor_mul(m2, G[f"C2{i}"], G["r2"])
        eng.tensor_add(m1, m1, m2)
        eng.tensor_mul(out_int[:, :, i], m1, G[f"Krd{i}"])

    nc.sync.dma_start(out=out_d.rearrange("(p f) c -> p f c", p=128), in_=out_int)


def build():
    """Build the per-core Bass program (same program for all 8 cores)."""
    from contextlib import ExitStack

    import concourse.bacc as bacc
    import concourse.tile as tile

    nc = bacc.Bacc(trn_type="TRN2", target_bir_lowering=False, debug=False)
    with tile.TileContext(nc) as tc:
        with ExitStack() as ctx:
            _emit(nc, tc, ctx)
    nc.compile()
    return nc


_NC_CACHE = []


def _shard_inputs(inputs):
    f32 = np.float32
    fmm = {"f16": np.float16, "f32r": f32, "f32": f32}[MM_DTYPE]
    q = np.ascontiguousarray(np.asarray(inputs["q"], dtype=f32))
    s = np.ascontiguousarray(np.asarray(inputs["s"], dtype=f32))
    sdd = np.asarray(inputs["s_Ddot"], dtype=f32)
    weights = {}
    for k in ("W0", "W1", "W2", "W3"):
        weights[k] = np.ascontiguousarray(np.asarray(inputs[k], dtype=f32).astype(fmm))
    for k in ("b0", "b1", "b2", "b3"):
        weights[k] = np.ascontiguousarray(np.asarray(inputs[k], dtype=f32))
    in_maps = []
    for c in range(N_CORES):
        sl = slice(c * BC, (c + 1) * BC)
        m = {
            "q": q[sl],
            "s": s[sl],
            "sddT": np.ascontiguousarray(sdd[sl].T).astype(fmm),
        }
        m.update(weights)
        in_maps.append(m)
    return in_maps


def kernel(**inputs) -> np.ndarray:
    from concourse import bass_utils

    if not _NC_CACHE:
        _NC_CACHE.append(build())
    nc = _NC_CACHE[0]

    in_maps = _shard_inputs(inputs)
    res = bass_utils.run_bass_kernel_spmd(nc, in_maps, core_ids=list(range(N_CORES)))
    out = np.concatenate([res.results[c]["out"] for c in range(N_CORES)], axis=0)
    return out.reshape(B_FULL, 3, 1).astype(np.float32)


if __name__ == "__main__":
    nc = build()
    print("built OK")


# revision 40
# speedup vs baseline: 1.1304x; 1.0187x over previous
"""Trainium2 Bass kernel for nn_B_NNs_34789235097695.

Problem: per batch element b (B=262144):
    y   = MLP(s_Ddot[b])  (3 -> 128 -> 128 -> 128 -> 3, tanh, fp32)
    K   = diag geometry from (q[b], s[b])
    A   = 3x3 geometry matrix from (q[b], s[b])
    out = Kdiag * solve(A, y + b3)        -> [B, 3, 1]

Strategy (8 cores, pure data parallel, 32768 batch rows per core):
  - ScalarE (ACT) is the floor: 3 tanh layers x 128 hidden x 32768 rows =
    12.6M tanh/core at 1 elem/lane/cycle @ 1.2 GHz (~85us + instr overhead).
    Everything else is arranged to hide under a continuous tanh stream.
  - MLP on PE in "hidden-on-partitions" layout (psum = W^T @ xT), float16
    operand chain (1 cyc/row; fp32 PSUM accumulate — measured end-to-end
    error identical to the all-fp32 envelope), chunks of 1024 batch columns.
  - 3-stage skewed software pipeline across chunks: the strict-FIFO ACT
    queue runs T0(i), T1(i-1), T2(i-2) so the PE work between dependent
    tanhs of one chunk hides under other chunks' tanhs. PSUM: 3 stage tiles
    (2 banks each) + 2 layer-3 tiles = 8 banks exactly.
  - PE warm-up burst + early filler matmuls keep the HAM clock gate at
    2.4 GHz (a cold PE at 1.2 GHz would out-run the ACT stream's slack).
  - Layer 3: W3 [128,3] stationary (3-column weight load), out [3,512]
    PSUM, DVE copy to a 4-chunk staging row, then one SBUF->SBUF DMA per
    component respreads to batch-on-partitions "order B" (b = p*256 + f).
  - Geometry (sin/cos polynomials — q in [0,1) — Kdiag, A) on VectorE;
    the pure tensor_tensor cofactor/det chain on the idle GpSimd engine;
    reciprocal pinned after the steady-state stream (add_dep_helper) so a
    wait on GpSimd can never head-of-line-block the DVE copy stream.
  - q/s/out move as 3KB-contiguous runs in order B; s_Ddot is transposed
    host-side during sharding so layer-0 rhs loads are 3 big runs/chunk.

Self-contained: hardcodes all shapes; needs only /opt/trn_rl_repo (the
container's Bass runtime) and the axon-tunneled NeuronCores.
"""

import sys

for _p in ("/opt/trn_rl_repo", "/root/.axon_site/_ro/trn_rl_repo"):
    if _p not in sys.path:
        sys.path.append(_p)

import numpy as np

B_FULL = 262144
N_CORES = 8
BC = B_FULL // N_CORES          # 32768 batch rows per core
F = BC // 128                   # 256 free columns in geometry layout
H = 128

RB = 0.06                       # BASE_RADIUS
RE = 0.045                      # END_EFFECTOR_RADIUS
LA = 0.176                      # LOWER_ARM_LENGTH

MM_DTYPE = "f16"                # "f16" (1 cyc/row) | "f32r" | "f32"

_alpha = np.deg2rad(np.array([-30.0, 90.0, 210.0], np.float32))
CA = [float(v) for v in np.cos(_alpha)]
SA = [float(v) for v in np.sin(_alpha)]

# sin (odd, t=x^2): c1..c9 ; cos (even): d0..d5   -- for q in [0, 1)
_SC = [1.0, -1.0 / 6, 1.0 / 120, -1.0 / 5040, 1.0 / 362880]
_CC = [1.0, -0.5, 1.0 / 24, -1.0 / 720, 1.0 / 40320, -1.0 / 3628800]


CHUNK = 1024                    # 2 PSUM banks per stage tile


def _chunks():
    assert BC % CHUNK == 0
    return [(i * CHUNK, CHUNK) for i in range(BC // CHUNK)]


def _emit(nc, tc, ctx):
    import concourse.bass as bass
    from concourse import mybir

    f32 = mybir.dt.float32
    ALU = mybir.AluOpType
    ACTF = mybir.ActivationFunctionType

    # dtype used along the matmul operand chain
    fmm = {
        "f16": mybir.dt.float16,
        "f32r": mybir.dt.float32r,
        "f32": f32,
    }[MM_DTYPE]

    # ---------------- DRAM tensors (per-core shapes) ----------------
    q_d = nc.dram_tensor("q", [BC, 3], f32, kind="ExternalInput").ap()
    s_d = nc.dram_tensor("s", [BC, 3], f32, kind="ExternalInput").ap()
    sddT_d = nc.dram_tensor("sddT", [3, BC], fmm, kind="ExternalInput").ap()
    W_d = [
        nc.dram_tensor("W0", [3, H], fmm, kind="ExternalInput").ap(),
        nc.dram_tensor("W1", [H, H], fmm, kind="ExternalInput").ap(),
        nc.dram_tensor("W2", [H, H], fmm, kind="ExternalInput").ap(),
        nc.dram_tensor("W3", [H, 3], fmm, kind="ExternalInput").ap(),
    ]
    b_d = [
        nc.dram_tensor("b0", [H], f32, kind="ExternalInput").ap(),
        nc.dram_tensor("b1", [H], f32, kind="ExternalInput").ap(),
        nc.dram_tensor("b2", [H], f32, kind="ExternalInput").ap(),
        nc.dram_tensor("b3", [3], f32, kind="ExternalInput").ap(),
    ]
    out_d = nc.dram_tensor("out", [BC, 3], f32, kind="ExternalOutput").ap()

    # ---------------- pools ----------------
    singles = ctx.enter_context(tc.tile_pool(name="singles", bufs=1))
    geo = ctx.enter_context(tc.tile_pool(name="geo", bufs=1))
    pool_in = ctx.enter_context(tc.tile_pool(name="pool_in", bufs=4))
    pool_h = ctx.enter_context(tc.tile_pool(name="pool_h", bufs=8))
    pool_stg = ctx.enter_context(tc.tile_pool(name="pool_stg", bufs=3))
    # 3 stage tags (mm0/mm1/mm2) x 1 buf x 2 banks + l3 2 x 1 bank = 8 banks
    psum_mm = ctx.enter_context(tc.tile_pool(name="psum_mm", bufs=1, space="PSUM"))
    psum_l3 = ctx.enter_context(tc.tile_pool(name="psum_l3", bufs=2, space="PSUM"))

    # ---------------- PE warm-up burst ----------------
    # HAM un-throttles the PE (1.2 -> 2.4 GHz) only after a ~3.4us window of
    # dense activity; once warm, the steady pipeline keeps it warm. Burn a
    # few us of dummy matmuls during the input-DMA prologue so the real
    # stream starts at full clock.
    warm = singles.tile([128, 128], fmm, name="warm", tag="warm")
    nc.vector.memset(warm, 0.0)
    wpsum = psum_l3.tile([3, 512], f32, name="wpsum", tag="l3")
    for _ in range(32):
        nc.tensor.matmul(wpsum[:, 0:128], warm[:, 0:3], warm,
                         start=True, stop=True)
    # dummy activation pulls the ~1.3us tanh ACT_TABLE_LOAD into the prologue
    wact = singles.tile([128, 1], f32, name="wact", tag="wact")
    nc.scalar.activation(wact, warm[:, 0:1], ACTF.Tanh)

    # ---------------- constants / weights in SBUF ----------------
    # Critical-path first: w0 + b0 + the first input chunk feed the first
    # tanh; everything else hides under the pipeline.
    w_sb = [None] * 4
    b_sb = [None] * 3

    def load_w(i):
        w = singles.tile(list(W_d[i].shape), W_d[i].dtype, name=f"w{i}sb",
                         tag=f"w{i}sb")
        nc.sync.dma_start(out=w, in_=W_d[i])
        w_sb[i] = w

    def load_b(i):
        b = singles.tile([H, 1], f32, name=f"b{i}sb", tag=f"b{i}sb")
        nc.sync.dma_start(out=b, in_=b_d[i].rearrange("(p one) -> p one", one=1))
        b_sb[i] = b

    load_w(0)
    load_b(0)

    # b3 broadcast to all partitions: [128, 3]
    b3bc = singles.tile([128, 3], f32, name="b3bc", tag="b3bc")

    def load_rest():
        for i in (1, 2, 3):
            load_w(i)
            if i < 3:
                load_b(i)
        nc.gpsimd.dma_start(
            out=b3bc,
            in_=bass.AP(tensor=b_d[3].tensor, offset=0, ap=[[0, 128], [1, 3]]),
        )

    # interleaved q/s in order B: partition p holds rows [p*F, (p+1)*F).
    # Loads are deferred into the geometry stream so they don't delay the
    # first MLP chunk's input DMA (ACT prologue latency).
    iq = singles.tile([128, F, 3], f32, name="iq", tag="iq")
    is_ = singles.tile([128, F, 3], f32, name="is_", tag="is_")

    # MLP output in order B, comp-major: yB[p, 256*c + f] = y[p*256 + f, c]
    yB = singles.tile([128, 3 * F], f32, name="yB", tag="yB")

    # ---------------- geometry op list (drained between chunks) ----------
    G = {}  # name -> AP

    def gt(name):
        t = geo.tile([128, F], f32, name=name, tag=name)
        G[name] = t
        return t

    geo_ops = []

    def deferred(fn):
        geo_ops.append(fn)

    vec = nc.vector

    def op_load_iq():
        # GpSimd SWDGE queue — keeps the Sync HWDGE queue free for the
        # latency-critical per-chunk input loads.
        nc.gpsimd.dma_start(out=iq, in_=q_d.rearrange("(p f) c -> p f c", p=128))

    def op_load_is():
        nc.gpsimd.dma_start(out=is_, in_=s_d.rearrange("(p f) c -> p f c", p=128))

    deferred(op_load_iq)
    deferred(op_load_is)

    def emit_trig(c):
        x = iq[:, :, c]
        gp = vec

        def op_t():
            t = gt(f"t{c}")
            gp.tensor_mul(t, x, x)
        deferred(op_t)

        def op_sin():
            t = G[f"t{c}"]
            c1, c3, c5, c7, c9 = _SC
            w = gt(f"sw{c}")
            gp.scalar_tensor_tensor(w, t, c7 / c9, t, op0=ALU.add, op1=ALU.mult)
            gp.scalar_tensor_tensor(w, w, c5 / c9, t, op0=ALU.add, op1=ALU.mult)
            gp.scalar_tensor_tensor(w, w, c3 / c9, t, op0=ALU.add, op1=ALU.mult)
            gp.tensor_scalar(w, w, c9, 1.0, op0=ALU.mult, op1=ALU.add)
            sq = gt(f"sq{c}")
            gp.tensor_mul(sq, w, x)
        deferred(op_sin)

        def op_cos():
            t = G[f"t{c}"]
            d0, d1, d2, d3, d4, d5 = _CC
            w = gt(f"cw{c}")
            gp.scalar_tensor_tensor(w, t, d4 / d5, t, op0=ALU.add, op1=ALU.mult)
            gp.scalar_tensor_tensor(w, w, d3 / d5, t, op0=ALU.add, op1=ALU.mult)
            gp.scalar_tensor_tensor(w, w, d2 / d5, t, op0=ALU.add, op1=ALU.mult)
            gp.scalar_tensor_tensor(w, w, d1 / d5, t, op0=ALU.add, op1=ALU.mult)
            cq = gt(f"cq{c}")
            gp.tensor_scalar(cq, w, d5, 1.0, op0=ALU.mult, op1=ALU.add)
        deferred(op_cos)

    def emit_kdiag_a(c):
        s0, s1, s2 = is_[:, :, 0], is_[:, :, 1], is_[:, :, 2]
        gp = vec

        def op_k():
            sq, cq = G[f"sq{c}"], G[f"cq{c}"]
            u = gt(f"ku{c}")
            gp.tensor_scalar(u, s0, CA[c], RB - RE, op0=ALU.mult, op1=ALU.add)
            gp.scalar_tensor_tensor(u, s1, SA[c], u, op0=ALU.mult, op1=ALU.add)
            gp.tensor_mul(u, u, sq)
            w = gt(f"kw{c}")
            gp.tensor_mul(w, s2, cq)
            k = gt(f"K{c}")
            gp.tensor_sub(k, u, w)
        deferred(op_k)

        def op_a():
            cq = G[f"cq{c}"]
            dR = RE - RB
            a0 = gt(f"a0{c}")
            gp.tensor_scalar(a0, cq, -LA * CA[c], dR * CA[c],
                             op0=ALU.mult, op1=ALU.add)
            gp.tensor_add(a0, a0, s0)
            a1 = gt(f"a1{c}")
            gp.tensor_scalar(a1, cq, -LA * SA[c], dR * SA[c],
                             op0=ALU.mult, op1=ALU.add)
            gp.tensor_add(a1, a1, s1)
            a2 = gt(f"a2{c}")
            gp.scalar_tensor_tensor(a2, cq, -LA, s2, op0=ALU.mult, op1=ALU.add)
        deferred(op_a)

    for c in range(3):
        emit_trig(c)
    for c in range(3):
        emit_kdiag_a(c)

    # cofactors C[i][j] of entry (i,j); adj = C^T ; x_i = sum_j C[j][i]*r_j
    COF = [
        ((0, 0), (1, 1), (2, 2), (1, 2), (2, 1)),
        ((0, 1), (1, 2), (2, 0), (1, 0), (2, 2)),
        ((0, 2), (1, 0), (2, 1), (1, 1), (2, 0)),
        ((1, 0), (0, 2), (2, 1), (0, 1), (2, 2)),
        ((1, 1), (0, 0), (2, 2), (0, 2), (2, 0)),
        ((1, 2), (0, 1), (2, 0), (0, 0), (2, 1)),
        ((2, 0), (0, 1), (1, 2), (0, 2), (1, 1)),
        ((2, 1), (0, 2), (1, 0), (0, 0), (1, 2)),
        ((2, 2), (0, 0), (1, 1), (0, 1), (1, 0)),
    ]

    # cofactors on the otherwise-idle GpSimd engine (SBUF-only elementwise)
    def emit_cof(spec):
        (ci, cj), (pi, pj), (pk, pl), (ni, nj), (nk, nl) = spec

        def op():
            gp = nc.gpsimd
            m1 = gt(f"cm1_{ci}{cj}")
            gp.tensor_mul(m1, G[f"a{pi}{pj}"], G[f"a{pk}{pl}"])
            m2 = gt(f"cm2_{ci}{cj}")
            gp.tensor_mul(m2, G[f"a{ni}{nj}"], G[f"a{nk}{nl}"])
            cc = gt(f"C{ci}{cj}")
            gp.tensor_sub(cc, m1, m2)
        deferred(op)

    for spec in COF:
        emit_cof(spec)

    def op_det():
        # GpSimd, like the cofactors it depends on — keeps the strict-FIFO
        # DVE queue free of cross-engine waits that would block the L3
        # staging copies behind them.
        gp = nc.gpsimd
        m1 = gt("dm1")
        gp.tensor_mul(m1, G["a00"], G["C00"])
        m2 = gt("dm2")
        gp.tensor_mul(m2, G["a01"], G["C01"])
        gp.tensor_add(m1, m1, m2)
        gp.tensor_mul(m2, G["a02"], G["C02"])
        det = gt("det")
        gp.tensor_add(det, m1, m2)
    deferred(op_det)

    LATE_COPY = []

    def op_rdet():
        # emitted after the chunk loop; the explicit (non-sync) dep edge on a
        # late L3 staging copy stops the scheduler from hoisting the
        # reciprocal into the steady-state DVE stream, where a wait on the
        # GpSimd det chain would head-of-line-block the PSUM copies.
        from concourse.tile import add_dep_helper

        rdet = gt("rdet")
        ri = vec.reciprocal(rdet, G["det"])
        if LATE_COPY:
            add_dep_helper(ri.ins, LATE_COPY[0].ins, sync=False,
                           reason="rdet after steady-state copies")
        for c in range(3):
            krd = gt(f"Krd{c}")
            vec.tensor_mul(krd, G[f"K{c}"], rdet)

    # ---------------- MLP chunks: 3-stage skewed software pipeline -------
    # ACT is the bottleneck engine and its queue is strict-FIFO, so tanh
    # instructions are emitted in the order T0(i), T1(i-1), T2(i-2): the
    # PE work between dependent tanhs of one chunk is hidden under the
    # other chunks' tanhs, keeping ACT (and PE, for HAM warmth) dense.
    chunks = _chunks()
    n_chunks = len(chunks)
    n_iters = n_chunks + 2
    pace = 12
    per_gap = (len(geo_ops) + pace - 1) // pace

    PS = {}   # (stage, chunk) -> psum tile
    HT = {}   # (stage, chunk) -> h tile

    def st_dma(ci):
        off, S = chunks[ci]
        sddc = pool_in.tile([3, S], fmm, name=f"sdd_{ci}", tag="sdd")
        nc.sync.dma_start(out=sddc, in_=sddT_d[:, off:off + S])
        HT[("x", ci)] = sddc

    def st_mm(layer, ci):
        _, S = chunks[ci]
        nS = S // 512
        src = HT[("x", ci)] if layer == 0 else HT[(layer - 1, ci)]
        ps = psum_mm.tile([128, S], f32, name=f"ps{layer}_{ci}",
                          tag=f"mm{layer}")
        if ci < 6:
            # HAM keep-warm fillers during pipeline fill: harmless garbage,
            # fully overwritten by the start=True matmuls below.
            for _ in range(3):
                nc.tensor.matmul(ps[0:3, 0:128], warm[:, 0:3], warm,
                                 start=True, stop=True)
        for k in range(nS):
            nc.tensor.matmul(ps[:, 512 * k:512 * (k + 1)], w_sb[layer],
                             src[:, 512 * k:512 * (k + 1)],
                             start=True, stop=True)
        PS[(layer, ci)] = ps

    def st_tanh(layer, ci):
        _, S = chunks[ci]
        h = pool_h.tile([128, S], fmm, name=f"h{layer}_{ci}", tag="h")
        nc.scalar.activation(h, PS[(layer, ci)], ACTF.Tanh, bias=b_sb[layer])
        HT[(layer, ci)] = h
        del PS[(layer, ci)]

    GRP = 4                       # chunks per staging group / reshape DMA
    STG = {}

    def st_l3(ci):
        off, S = chunks[ci]
        nS = S // 512
        g = ci // GRP
        if g not in STG:
            STG[g] = pool_stg.tile([3, GRP * CHUNK], f32, name=f"stg_{g}",
                                   tag="stg")
        stg = STG[g]
        goff = (ci % GRP) * CHUNK
        h3 = HT[(2, ci)]
        for k in range(nS):
            psl3 = psum_l3.tile([3, 512], f32, name=f"l3_{ci}_{k}", tag="l3")
            nc.tensor.matmul(psl3, w_sb[3], h3[:, 512 * k:512 * (k + 1)],
                             start=True, stop=True)
            cp = vec.tensor_copy(stg[:, goff + 512 * k:goff + 512 * (k + 1)],
                                 psl3)
            if ci == 26 and k == 0:
                LATE_COPY.append(cp)
        if ci % GRP == GRP - 1:
            nP = GRP * CHUNK // F
            p0 = (off + S - GRP * CHUNK) // F
            for c in range(3):
                nc.sync.dma_start(
                    out=yB[p0:p0 + nP, F * c:F * (c + 1)],
                    in_=stg[c:c + 1, :].rearrange("one (p f) -> one p f", f=F),
                )
            del STG[g]

    st_dma(0)
    load_rest()
    st_mm(0, 0)
    st_dma(1)
    for i in range(n_iters):
        if i + 2 < n_chunks:
            st_dma(i + 2)
        if i + 1 < n_chunks:
            st_mm(0, i + 1)
        if i < n_chunks:
            st_tanh(0, i)
            st_mm(1, i)
        if 0 <= i - 1 < n_chunks:
            st_tanh(1, i - 1)
            st_mm(2, i - 1)
        if 0 <= i - 2 < n_chunks:
            st_tanh(2, i - 2)
            st_l3(i - 2)
        for _ in range(per_gap):
            if geo_ops:
                geo_ops.pop(0)()

    while geo_ops:
        geo_ops.pop(0)()
    op_rdet()

    # ---------------- r_c = yB_c + b3[c] --------------------------------
    # tensor_scalar on GpSimd is an emulated/slow path (~4us) — keep all
    # three adds on DVE; only plain tensor_tensor work goes to GpSimd.
    for c in range(3):
        rb = gt(f"r{c}")
        vec.tensor_scalar(rb, yB[:, F * c:F * (c + 1)], b3bc[:, c:c + 1],
                          None, op0=ALU.add)

    # ---------------- final combine: out = Krd * (C^T r) ----------------
    out_int = singles.tile([128, F, 3], f32, name="out_int", tag="out_int")
    for i in range(3):
        eng = nc.gpsimd if i == 2 else vec
        m1 = gt(f"fm1_{i}")
        eng.tensor_mul(m1, G[f"C0{i}"], G["r0"])
        m2 = gt(f"fm2_{i}")
        eng.tensor_mul(m2, G[f"C1{i}"], G["r1"])
        eng.tensor_add(m1, m1, m2)
        eng.tensor_mul(m2, G[f"C2{i}"], G["r2"])
        eng.tensor_add(m1, m1, m2)
        eng.tensor_mul(out_int[:, :, i], m1, G[f"Krd{i}"])

    nc.sync.dma_start(out=out_d.rearrange("(p f) c -> p f c", p=128), in_=out_int)


def build():
    """Build the per-core Bass program (same program for all 8 cores)."""
    from contextlib import ExitStack

    import concourse.bacc as bacc
    import concourse.tile as tile

    nc = bacc.Bacc(trn_type="TRN2", target_bir_lowering=False, debug=False)
    with tile.TileContext(nc) as tc:
        with ExitStack() as ctx:
            _emit(nc, tc, ctx)
    nc.compile()
    return nc


_NC_CACHE = []


def _shard_inputs(inputs):
    f32 = np.float32
    fmm = {"f16": np.float16, "f32r": f32, "f32": f32}[MM_DTYPE]
    q = np.ascontiguousarray(np.asarray(inputs["q"], dtype=f32))
    s = np.ascontiguousarray(np.asarray(inputs["s"], dtype=f32))
    sdd = np.asarray(inputs["s_Ddot"], dtype=f32)
    weights = {}
    for k in ("W0", "W1", "W2", "W3"):
        weights[k] = np.ascontiguousarray(np.asarray(inputs[k], dtype=f32).astype(fmm))
    for k in ("b0", "b1", "b2", "b3"):
        weights[k] = np.ascontiguousarray(np.asarray(inputs[k], dtype=f32))
    in_maps = []
    for c in range(N_CORES):
        sl = slice(c * BC, (c + 1) * BC)
        m = {
            "q": q[sl],
            "s": s[sl],
            "sddT": np.ascontiguousarray(sdd[sl].T).astype(fmm),
        }
        m.update(weights)
        in_maps.append(m)
    return in_maps


def kernel(**inputs) -> np.ndarray:
    from concourse import bass_utils

    if not _NC_CACHE:
        _NC_CACHE.append(build())
    nc = _NC_CACHE[0]

    in_maps = _shard_inputs(inputs)
    res = bass_utils.run_bass_kernel_spmd(nc, in_maps, core_ids=list(range(N_CORES)))
    out = np.concatenate([res.results[c]["out"] for c in range(N_CORES)], axis=0)
    return out.reshape(B_FULL, 3, 1).astype(np.float32)


if __name__ == "__main__":
    nc = build()
    print("built OK")
